# revision 23
# baseline (speedup 1.0000x reference)
"""Multi-head causal attention (B=4, T=2048, D=1024, H=16) on 8 NeuronCores.

Sharding: data-parallel over batch (4) x tensor-parallel over heads (2 groups
of 8 heads) = 8 cores, with NO collectives. Each core runs the QKV projection
for its head group (bf16 weights/activations, fp32 psum), causal
flash-attention for its 8 heads over the full sequence, and a PARTIAL output
projection y_g = O_g^T @ W_out[rows of g] over the full sequence. The host
adds the two partials per batch (fp32) - the only cross-core combination.
Removing the ReduceScatter makes every core's timeline independent (no
dispatch-skew amplification through collective sync) and drops the exchange
DMA round-trips.

Scores are computed transposed, S^T[s, tq], so the softmax normalizer comes
free from a ones-column appended to V, and the 1/sqrt(dh) score scale folds
into the ACT exp's scale argument. Only lower-triangle score blocks are
computed. The two heads of a pair share one [128, 1024] PSUM score tile
(head0 cols 0:512, head1 cols 512:1024) so each s-block needs a single exp
instruction; on diagonal blocks a strided 3D AP narrows both halves in one
instruction. AV matmuls are software-pipelined one s-block behind the exp so
the PE never waits on ACT, and QK-projection / output-projection matmuls for
other chunks are interleaved between attention units as PE filler while ACT
drains. Output chunks DMA directly from PSUM on the ACT hwdge queue (inputs
use the sync queue) to avoid head-of-line blocking.
"""
import sys

sys.path.insert(0, '/opt/trn_rl_repo')

import numpy as np

import concourse.mybir as mybir
import concourse.tile as tile
from concourse import bacc
from concourse import bass as bass_module
from concourse.bass_utils import run_bass_kernel_spmd

B, T, D = 4, 2048, 1024
H, DH = 16, 64
HG = 8              # heads per core
GD = HG * DH        # 512 features per core
P = 128
CH = 512            # tq chunk width (one psum bank)
NB = T // P         # 16 s-blocks
NCH = T // CH       # 4 tq chunks
KB = D // P         # 8 contraction blocks over d_model
NPAIR = HG // 2     # 4 head pairs per core
F32 = mybir.dt.float32
BF16 = mybir.dt.bfloat16
EXP = mybir.ActivationFunctionType.Exp
SCALE = float(DH ** -0.5)

_CACHE = {}


def build_nc(repeat=1):
    nc = bacc.Bacc("TRN2", target_bir_lowering=False, debug=False)

    xT = nc.dram_tensor("xT", [D, T], BF16, kind="ExternalInput")
    wq = nc.dram_tensor("wq", [D, GD], BF16, kind="ExternalInput")
    wk = nc.dram_tensor("wk", [D, GD], BF16, kind="ExternalInput")
    wv = nc.dram_tensor("wv", [D, GD], BF16, kind="ExternalInput")
    wout = nc.dram_tensor("wout", [GD, D], BF16, kind="ExternalInput")
    tril = nc.dram_tensor("tril", [P, P], BF16, kind="ExternalInput")
    ident = nc.dram_tensor("ident", [P, P], BF16, kind="ExternalInput")
    y = nc.dram_tensor("y", [T, D], F32, kind="ExternalOutput")

    with tile.TileContext(nc) as tc:
        with tc.tile_pool(name="cst", bufs=1) as cst, \
             tc.tile_pool(name="wres", bufs=1) as wres, \
             tc.tile_pool(name="big", bufs=1) as big, \
             tc.tile_pool(name="qk", bufs=1) as qkp, \
             tc.tile_pool(name="vp", bufs=1) as vput, \
             tc.tile_pool(name="oall", bufs=1) as oall, \
             tc.tile_pool(name="exps", bufs=4) as expp, \
             tc.tile_pool(name="sml", bufs=2) as sml, \
             tc.tile_pool(name="ystg", bufs=3) as ystg, \
             tc.tile_pool(name="ps_a", bufs=2, space="PSUM") as ps_a, \
             tc.tile_pool(name="ps_s", bufs=2, space="PSUM") as ps_s, \
             tc.tile_pool(name="ps_o", bufs=1, space="PSUM") as ps_o:

            tril_sb = cst.tile([P, P], BF16)
            id_sb = cst.tile([P, P], BF16)

            # Resident weights. wv/xT interleaved per k-block so the first
            # V-projection matmul can start after ~2 DMAs.
            wv_sb = wres.tile([P, KB, GD], BF16, tag="wv")
            wv_r = wv.ap().rearrange("(ko p) n -> p ko n", p=P)
            wq_sb = wres.tile([P, KB, GD], BF16, tag="wq")
            wq_r = wq.ap().rearrange("(ko p) n -> p ko n", p=P)
            wk_sb = wres.tile([P, KB, GD], BF16, tag="wk")
            wk_r = wk.ap().rearrange("(ko p) n -> p ko n", p=P)
            wo_sb = wres.tile([P, NPAIR, D], BF16, tag="wo")
            wo_r = wout.ap().rearrange("(fo p) n -> p fo n", p=P)

            xT_sb = big.tile([P, KB, T], BF16, tag="big")
            xT_r = xT.ap().rearrange("(ko p) t -> p ko t", p=P)

            v_aug = vput.tile([P, NB, NPAIR, 2 * (DH + 1)], BF16, tag="va")
            nc.vector.memset(v_aug[:, :, :, DH:DH + 1], 1.0)
            nc.vector.memset(v_aug[:, :, :, 2 * DH + 1:], 1.0)

            qt = qkp.tile([P, NPAIR, T], BF16, tag="qt")
            kt = qkp.tile([P, NPAIR, T], BF16, tag="kt")
            oT_sb = oall.tile([P, NPAIR, T], BF16, tag="oall")

            for _rep in range(repeat):
                # xT split per (k, t-chunk), chunk 0 first: the first V-proj
                # s-blocks and QK(c4=0) become runnable after ~1MB of DMA
                # instead of the full 4MB.
                if _rep == 0:
                    for k in range(KB):
                        nc.sync.dma_start(wv_sb[:, k], wv_r[:, k])
                        nc.sync.dma_start(xT_sb[:, k, 0:CH], xT_r[:, k, 0:CH])
                        if k == 1:
                            nc.sync.dma_start(tril_sb[:], tril.ap())
                            nc.sync.dma_start(id_sb[:], ident.ap())
                    for c4 in range(1, NCH):
                        for k in range(KB):
                            nc.sync.dma_start(
                                xT_sb[:, k, c4 * CH:(c4 + 1) * CH],
                                xT_r[:, k, c4 * CH:(c4 + 1) * CH])
                    for k in range(0, KB, 2):
                        nc.sync.dma_start(wq_sb[:, k:k + 2], wq_r[:, k:k + 2])
                        nc.sync.dma_start(wk_sb[:, k:k + 2], wk_r[:, k:k + 2])
                    for f in range(0, NPAIR, 2):
                        nc.sync.dma_start(wo_sb[:, f:f + 2], wo_r[:, f:f + 2])
                else:
                    for c4 in range(NCH):
                        for k in range(KB):
                            nc.sync.dma_start(
                                xT_sb[:, k, c4 * CH:(c4 + 1) * CH],
                                xT_r[:, k, c4 * CH:(c4 + 1) * CH])

                # ---- V projection; per pair: [V0 | 1 | V1 | 1] columns so
                # the AV matmul lhsT slice [h2*65 : h2*65+65] puts O at psum
                # partitions 0..63 and the softmax denominator at 64.
                for sb_i in range(NB):
                    psum = ps_a.tile([P, CH], F32, tag="proj")
                    for k in range(KB):
                        nc.tensor.matmul(
                            psum[:],
                            lhsT=xT_sb[:, k, sb_i * P:(sb_i + 1) * P],
                            rhs=wv_sb[:, k, :],
                            start=(k == 0), stop=(k == KB - 1),
                        )
                    ps_r = psum.rearrange("p (pr h2 d) -> p pr h2 d",
                                          pr=NPAIR, h2=2)
                    nc.vector.tensor_copy(
                        out=v_aug[:, sb_i, :, 0:DH], in_=ps_r[:, :, 0, :])
                    nc.vector.tensor_copy(
                        out=v_aug[:, sb_i, :, DH + 1:2 * DH + 1],
                        in_=ps_r[:, :, 1, :])

                def qk_unit(c4, hh, w_sb, dst):
                    psum = ps_a.tile([P, CH], F32, tag="proj")
                    for k in range(KB):
                        nc.tensor.matmul(
                            psum[:],
                            lhsT=w_sb[:, k, hh * P:(hh + 1) * P],
                            rhs=xT_sb[:, k, c4 * CH:(c4 + 1) * CH],
                            start=(k == 0), stop=(k == KB - 1),
                        )
                    nc.vector.tensor_copy(
                        out=dst[:, hh, c4 * CH:(c4 + 1) * CH], in_=psum[:])

                def op_unit(c, u):
                    tb, ncn = 4 * c + u // 2, u % 2
                    psum = ps_a.tile([P, CH], F32, tag="proj")
                    for fb in range(NPAIR):
                        nc.tensor.matmul(
                            psum[:],
                            lhsT=oT_sb[:, fb, tb * P:(tb + 1) * P],
                            rhs=wo_sb[:, fb, ncn * CH:(ncn + 1) * CH],
                            start=(fb == 0), stop=(fb == NPAIR - 1),
                        )
                    y_sb = ystg.tile([P, CH], F32, tag="y")
                    nc.vector.tensor_copy(out=y_sb[:], in_=psum[:])
                    nc.scalar.dma_start(
                        y.ap()[tb * P:(tb + 1) * P, ncn * CH:(ncn + 1) * CH],
                        y_sb[:])

                for hh in range(NPAIR):
                    qk_unit(0, hh, wq_sb, qt)
                    qk_unit(0, hh, wk_sb, kt)

                # ---- attention, chunk-major. `filler` is a queue of PE work
                # (later QK projections, earlier-chunk output projections)
                # pumped into the attention stream at fine grain so the PE
                # never idles while ACT drains exp backlogs. Queue order
                # guarantees QK(c) completes before attn(c) needs it.
                filler = []

                def pump(n):
                    for _ in range(n):
                        if filler:
                            filler.pop(0)()

                def attn_unit(c, hh, npump, last=False):
                    nblk = (c + 1) * (CH // P)
                    HW2 = CH // 2
                    # AV accumulators split by column half: otA covers tq
                    # cols [0:256] of the chunk and closes at s-block
                    # nblk-3 (later diagonal blocks only write cols >= 256),
                    # otB covers [256:512] and closes at nblk-1. Each holds
                    # both heads so a half is one PSUM bank.
                    otA = ps_o.tile([DH + 1, 2, HW2], F32, tag="otA")
                    otB = ps_o.tile([DH + 1, 2, HW2], F32, tag="otB")
                    # zero via DVE, then accumulate with start=False
                    # throughout: a start=True would clear the whole bank,
                    # wiping the other head sharing it.
                    nc.vector.memset(otA[:], 0.0)
                    nc.vector.memset(otB[:], 0.0)
                    pend = None

                    def flush_av(pend):
                        e_sb, i, f0 = pend
                        for h2 in range(2):
                            lhsT = v_aug[:, i, hh,
                                         h2 * (DH + 1):(h2 + 1) * (DH + 1)]
                            if f0 < HW2:
                                nc.tensor.matmul(
                                    otA[:, h2, f0:HW2],
                                    lhsT=lhsT,
                                    rhs=e_sb[:, h2 * CH + f0:h2 * CH + HW2],
                                    start=False,
                                    stop=(i == nblk - 3 and h2 == 1),
                                    skip_group_check=True,
                                )
                                nc.tensor.matmul(
                                    otB[:, h2, :],
                                    lhsT=lhsT,
                                    rhs=e_sb[:, h2 * CH + HW2:(h2 + 1) * CH],
                                    start=False,
                                    stop=(i == nblk - 1 and h2 == 1),
                                    skip_group_check=True,
                                )
                            else:
                                nc.tensor.matmul(
                                    otB[:, h2, f0 - HW2:HW2],
                                    lhsT=lhsT,
                                    rhs=e_sb[:, h2 * CH + f0:(h2 + 1) * CH],
                                    start=False,
                                    stop=(i == nblk - 1 and h2 == 1),
                                    skip_group_check=True,
                                )

                    # normalize, engine-interleaved h2 chains, h2=1 first
                    # (its partition-shifting SBUF->SBUF DMA gates readers of
                    # oT_sb); h2=0 writes straight into oT_sb (partitions
                    # 0-63 line up). Column-split: cols [0:384] are final one
                    # s-block early (the last diagonal block only writes
                    # [384:512]), so the bulk of the chain overlaps the last
                    # AV and only a narrow chain trails it.
                    recip = [sml.tile([1, CH], F32, tag="recip",
                                      name=f"recip{h2}")
                             for h2 in range(2)]
                    bc = [sml.tile([DH, CH], F32, tag="bc", name=f"bc{h2}")
                          for h2 in range(2)]

                    def emit_norm(tile, lo):
                        hi = lo + HW2
                        for h2 in (1, 0):
                            nc.vector.reciprocal(
                                recip[h2][:, lo:hi],
                                tile[DH:DH + 1, h2, :])
                        for h2 in (1, 0):
                            nc.gpsimd.partition_broadcast(
                                bc[h2][:, lo:hi], recip[h2][:, lo:hi])
                        o_n = sml.tile([DH, CH], BF16, tag="on")
                        nc.vector.tensor_mul(
                            out=o_n[:, lo:hi], in0=tile[0:DH, 1, :],
                            in1=bc[1][:, lo:hi])
                        nc.scalar.dma_start(
                            oT_sb[DH:P, hh, c * CH + lo:c * CH + hi],
                            o_n[:, lo:hi])
                        nc.vector.tensor_mul(
                            out=oT_sb[0:DH, hh, c * CH + lo:c * CH + hi],
                            in0=tile[0:DH, 0, :], in1=bc[0][:, lo:hi])

                    pump_at = set()
                    if npump:
                        for j in range(npump - 1):
                            pump_at.add(2 + (j * max(nblk - 2, 1))
                                        // max(npump - 1, 1))
                    for i in range(nblk):
                        r = i - c * (CH // P)
                        f0 = P * r if r >= 0 else 0
                        s_ps = ps_s.tile([P, 2 * CH], F32, tag="s")
                        e_sb = expp.tile([P, 2 * CH], BF16, tag="e")
                        for h2 in range(2):
                            pb = h2 * DH
                            nc.tensor.matmul(
                                s_ps[:, h2 * CH + f0:(h2 + 1) * CH],
                                lhsT=kt[pb:pb + DH, hh, i * P:(i + 1) * P],
                                rhs=qt[pb:pb + DH, hh,
                                       c * CH + f0:(c + 1) * CH],
                                start=True, stop=True,
                            )
                        if r >= 0:
                            for h2 in range(2):
                                nc.tensor.matmul(
                                    s_ps[:, h2 * CH + f0:h2 * CH + f0 + P],
                                    lhsT=id_sb[:],
                                    rhs=tril_sb[:],
                                    start=False, stop=True,
                                    skip_group_check=True,
                                )
                        # one exp for both heads: strided [P, 2, CH-f0]
                        s_v = s_ps.rearrange(
                            "p (h w) -> p h w", h=2)[:, :, f0:CH]
                        e_v = e_sb.rearrange(
                            "p (h w) -> p h w", h=2)[:, :, f0:CH]
                        nc.scalar.activation(e_v, s_v, EXP, scale=SCALE)
                        if pend is not None:
                            flush_av(pend)
                            if pend[1] == nblk - 3:
                                emit_norm(otA, 0)
                        if i in pump_at:
                            pump(1)
                        pend = (e_sb, i, f0)

                    pump(1)   # cover the last exp's latency
                    flush_av(pend)
                    if last:
                        pump(len(filler))
                    emit_norm(otB, HW2)

                # c=3 pumps only 3/unit so ~4 OP(2) units remain for the
                # final pump() below - they keep the PE busy during the last
                # unit's normalize chain that gates OP(3).
                NPUMP = {0: 2, 1: 2, 2: 4, 3: 3}
                for c in range(NCH):
                    if c + 1 < NCH:
                        for hh in range(NPAIR):
                            filler.append(
                                lambda c4=c + 1, hh=hh:
                                qk_unit(c4, hh, wq_sb, qt))
                            filler.append(
                                lambda c4=c + 1, hh=hh:
                                qk_unit(c4, hh, wk_sb, kt))
                    if c >= 1:
                        for u in range(2 * NPAIR):
                            filler.append(lambda cc=c - 1, u=u: op_unit(cc, u))
                    for hh in range(NPAIR):
                        attn_unit(c, hh, NPUMP[c],
                                  last=(c == NCH - 1 and hh == NPAIR - 1))

                pump(len(filler))
                for u in range(2 * NPAIR):
                    op_unit(NCH - 1, u)

    nc.compile()
    return nc


def _get_nc():
    if 'nc' not in _CACHE:
        _CACHE['nc'] = build_nc()
    return _CACHE['nc']


def _make_in_maps(inputs):
    import ml_dtypes
    bf16 = ml_dtypes.bfloat16
    x = np.asarray(inputs["x"], dtype=np.float32)
    W_qkv = np.asarray(inputs["W_qkv"], dtype=np.float32)
    W_out = np.asarray(inputs["W_out"], dtype=np.float32)

    tril_m = np.where(
        np.arange(P)[:, None] <= np.arange(P)[None, :], 0.0, -1e30
    ).astype(bf16)
    ident = np.eye(P, dtype=bf16)

    in_maps = []
    for core in range(8):
        b, g = core // 2, core % 2
        in_maps.append({
            "xT": np.ascontiguousarray(x[b].T).astype(bf16),
            "wq": np.ascontiguousarray(
                W_qkv[:, g * GD:(g + 1) * GD]).astype(bf16),
            "wk": np.ascontiguousarray(
                W_qkv[:, D + g * GD:D + (g + 1) * GD]).astype(bf16),
            "wv": np.ascontiguousarray(
                W_qkv[:, 2 * D + g * GD:2 * D + (g + 1) * GD]).astype(bf16),
            "wout": np.ascontiguousarray(
                W_out[g * GD:(g + 1) * GD, :]).astype(bf16),
            "tril": tril_m,
            "ident": ident,
        })
    return in_maps


def kernel(x, W_qkv, W_out, mask):
    """Full inputs in, full output out. mask is the known causal tril."""
    in_maps = _make_in_maps({"x": x, "W_qkv": W_qkv, "W_out": W_out})
    nc = _get_nc()
    try:
        res = run_bass_kernel_spmd(nc, in_maps, core_ids=list(range(8)))
    except Exception:
        import time as _time
        _time.sleep(30)   # transient axon-tunnel hiccups; one retry
        res = run_bass_kernel_spmd(nc, in_maps, core_ids=list(range(8)))

    out = np.empty((B, T, D), dtype=np.float32)
    for b in range(B):
        np.add(res.results[2 * b]["y"], res.results[2 * b + 1]["y"],
               out=out[b])
    return out


# revision 31
# speedup vs baseline: 1.7761x; 1.7761x over previous
"""Multi-head causal attention (B=4, T=2048, D=1024, H=16) on 8 NeuronCores.

Sharding: data-parallel over batch (4) x tensor-parallel over heads (2 groups
of 8 heads) = 8 cores, with NO collectives. Each core runs the QKV projection
for its head group (bf16 weights/activations, fp32 psum), causal
flash-attention for its 8 heads over the full sequence, and a PARTIAL output
projection y_g = O_g^T @ W_out[rows of g] over the full sequence. The host
adds the two partials per batch (fp32) - the only cross-core combination.
Removing the ReduceScatter makes every core's timeline independent (no
dispatch-skew amplification through collective sync) and drops the exchange
DMA round-trips.

Scores are computed transposed, S^T[s, tq], so the softmax normalizer comes
free from a ones-column appended to V, and the 1/sqrt(dh) score scale folds
into the ACT exp's scale argument. Only lower-triangle score blocks are
computed. The two heads of a pair share one [128, 1024] PSUM score tile
(head0 cols 0:512, head1 cols 512:1024) so each s-block needs a single exp
instruction; on diagonal blocks a strided 3D AP narrows both halves in one
instruction. AV matmuls are software-pipelined one s-block behind the exp so
the PE never waits on ACT, and QK-projection / output-projection matmuls for
other chunks are interleaved between attention units as PE filler while ACT
drains. Output chunks DMA directly from PSUM on the ACT hwdge queue (inputs
use the sync queue) to avoid head-of-line blocking.
"""
import sys

sys.path.insert(0, '/opt/trn_rl_repo')

import numpy as np

import concourse.mybir as mybir
import concourse.tile as tile
from concourse import bacc
from concourse import bass as bass_module
from concourse.bass_utils import run_bass_kernel_spmd

B, T, D = 4, 2048, 1024
H, DH = 16, 64
HG = 8              # heads per core
GD = HG * DH        # 512 features per core
P = 128
CH = 512            # tq chunk width (one psum bank)
NB = T // P         # 16 s-blocks
NCH = T // CH       # 4 tq chunks
KB = D // P         # 8 contraction blocks over d_model
NPAIR = HG // 2     # 4 head pairs per core
F32 = mybir.dt.float32
BF16 = mybir.dt.bfloat16
EXP = mybir.ActivationFunctionType.Exp
SCALE = float(DH ** -0.5)

_CACHE = {}


def build_nc(repeat=1, expp_bufs=6, sml_bufs=2, npump=None):
    nc = bacc.Bacc("TRN2", target_bir_lowering=False, debug=False)

    xT = nc.dram_tensor("xT", [D, T], BF16, kind="ExternalInput")
    wq = nc.dram_tensor("wq", [D, GD], BF16, kind="ExternalInput")
    wk = nc.dram_tensor("wk", [D, GD], BF16, kind="ExternalInput")
    wv = nc.dram_tensor("wv", [D, GD], BF16, kind="ExternalInput")
    wout = nc.dram_tensor("wout", [GD, D], BF16, kind="ExternalInput")
    tril = nc.dram_tensor("tril", [P, P], BF16, kind="ExternalInput")
    ident = nc.dram_tensor("ident", [P, P], BF16, kind="ExternalInput")
    y = nc.dram_tensor("y", [T, D], F32, kind="ExternalOutput")

    with tile.TileContext(nc) as tc:
        with tc.tile_pool(name="cst", bufs=1) as cst, \
             tc.tile_pool(name="wres", bufs=1) as wres, \
             tc.tile_pool(name="big", bufs=1) as big, \
             tc.tile_pool(name="qk", bufs=1) as qkp, \
             tc.tile_pool(name="vp", bufs=1) as vput, \
             tc.tile_pool(name="oall", bufs=1) as oall, \
             tc.tile_pool(name="exps", bufs=expp_bufs) as expp, \
             tc.tile_pool(name="sml", bufs=sml_bufs) as sml, \
             tc.tile_pool(name="ystg", bufs=3) as ystg, \
             tc.tile_pool(name="ps_a", bufs=2, space="PSUM") as ps_a, \
             tc.tile_pool(name="ps_s", bufs=2, space="PSUM") as ps_s, \
             tc.tile_pool(name="ps_o", bufs=1, space="PSUM") as ps_o:

            tril_sb = cst.tile([P, P], BF16)
            id_sb = cst.tile([P, P], BF16)

            # Resident weights. wv/xT interleaved per k-block so the first
            # V-projection matmul can start after ~2 DMAs.
            wv_sb = wres.tile([P, KB, GD], BF16, tag="wv")
            wv_r = wv.ap().rearrange("(ko p) n -> p ko n", p=P)
            wq_sb = wres.tile([P, KB, GD], BF16, tag="wq")
            wq_r = wq.ap().rearrange("(ko p) n -> p ko n", p=P)
            wk_sb = wres.tile([P, KB, GD], BF16, tag="wk")
            wk_r = wk.ap().rearrange("(ko p) n -> p ko n", p=P)
            wo_sb = wres.tile([P, NPAIR, D], BF16, tag="wo")
            wo_r = wout.ap().rearrange("(fo p) n -> p fo n", p=P)

            xT_sb = big.tile([P, KB, T], BF16, tag="big")
            xT_r = xT.ap().rearrange("(ko p) t -> p ko t", p=P)

            v_aug = vput.tile([P, NB, NPAIR, 2 * (DH + 1)], BF16, tag="va")
            nc.vector.memset(v_aug[:, :, :, DH:DH + 1], 1.0)
            nc.vector.memset(v_aug[:, :, :, 2 * DH + 1:], 1.0)

            qt = qkp.tile([P, NPAIR, T], BF16, tag="qt")
            kt = qkp.tile([P, NPAIR, T], BF16, tag="kt")
            oT_sb = oall.tile([P, NPAIR, T], BF16, tag="oall")

            for _rep in range(repeat):
                # xT split per (k, t-chunk), chunk 0 first: the first V-proj
                # s-blocks and QK(c4=0) become runnable after ~1MB of DMA
                # instead of the full 4MB.
                if _rep == 0:
                    for k in range(KB):
                        nc.sync.dma_start(wv_sb[:, k], wv_r[:, k])
                        nc.sync.dma_start(xT_sb[:, k, 0:CH], xT_r[:, k, 0:CH])
                        if k == 1:
                            nc.sync.dma_start(tril_sb[:], tril.ap())
                            nc.sync.dma_start(id_sb[:], ident.ap())
                    # wq/wk BEFORE the later xT chunks: QK(0) (which gates
                    # the first exps) only needs xT chunk 0 + these weights.
                    for k in range(0, KB, 2):
                        nc.sync.dma_start(wq_sb[:, k:k + 2], wq_r[:, k:k + 2])
                        nc.sync.dma_start(wk_sb[:, k:k + 2], wk_r[:, k:k + 2])
                    for c4 in range(1, NCH):
                        for k in range(KB):
                            nc.sync.dma_start(
                                xT_sb[:, k, c4 * CH:(c4 + 1) * CH],
                                xT_r[:, k, c4 * CH:(c4 + 1) * CH])
                    for f in range(0, NPAIR, 2):
                        nc.sync.dma_start(wo_sb[:, f:f + 2], wo_r[:, f:f + 2])
                else:
                    for c4 in range(NCH):
                        for k in range(KB):
                            nc.sync.dma_start(
                                xT_sb[:, k, c4 * CH:(c4 + 1) * CH],
                                xT_r[:, k, c4 * CH:(c4 + 1) * CH])

                # ---- V projection; per pair: [V0 | 1 | V1 | 1] columns so
                # the AV matmul lhsT slice [h2*65 : h2*65+65] puts O at psum
                # partitions 0..63 and the softmax denominator at 64.
                def v_unit(sb_i):
                    psum = ps_a.tile([P, CH], F32, tag="proj")
                    for k in range(KB):
                        nc.tensor.matmul(
                            psum[:],
                            lhsT=xT_sb[:, k, sb_i * P:(sb_i + 1) * P],
                            rhs=wv_sb[:, k, :],
                            start=(k == 0), stop=(k == KB - 1),
                        )
                    ps_r = psum.rearrange("p (pr h2 d) -> p pr h2 d",
                                          pr=NPAIR, h2=2)
                    nc.vector.tensor_copy(
                        out=v_aug[:, sb_i, :, 0:DH], in_=ps_r[:, :, 0, :])
                    nc.vector.tensor_copy(
                        out=v_aug[:, sb_i, :, DH + 1:2 * DH + 1],
                        in_=ps_r[:, :, 1, :])

                def qk_unit(c4, hh, w_sb, dst):
                    psum = ps_a.tile([P, CH], F32, tag="proj")
                    for k in range(KB):
                        nc.tensor.matmul(
                            psum[:],
                            lhsT=w_sb[:, k, hh * P:(hh + 1) * P],
                            rhs=xT_sb[:, k, c4 * CH:(c4 + 1) * CH],
                            start=(k == 0), stop=(k == KB - 1),
                        )
                    nc.vector.tensor_copy(
                        out=dst[:, hh, c4 * CH:(c4 + 1) * CH], in_=psum[:])

                def op_unit(c, u):
                    tb, ncn = 4 * c + u // 2, u % 2
                    psum = ps_a.tile([P, CH], F32, tag="proj")
                    for fb in range(NPAIR):
                        nc.tensor.matmul(
                            psum[:],
                            lhsT=oT_sb[:, fb, tb * P:(tb + 1) * P],
                            rhs=wo_sb[:, fb, ncn * CH:(ncn + 1) * CH],
                            start=(fb == 0), stop=(fb == NPAIR - 1),
                        )
                    y_sb = ystg.tile([P, CH], F32, tag="y")
                    nc.vector.tensor_copy(out=y_sb[:], in_=psum[:])
                    nc.scalar.dma_start(
                        y.ap()[tb * P:(tb + 1) * P, ncn * CH:(ncn + 1) * CH],
                        y_sb[:])

                # Only V s-blocks 0-3 + QK(0) before attention: the first
                # exps (the ACT bottleneck's window) start ~30us earlier;
                # the remaining V-projection units go through the filler
                # queue, paced so v_aug s-blocks land before their chunk.
                for sb_i in range(CH // P):
                    v_unit(sb_i)
                for hh in range(NPAIR):
                    qk_unit(0, hh, wq_sb, qt)
                    qk_unit(0, hh, wk_sb, kt)

                # ---- attention, chunk-major. `filler` is a queue of PE work
                # (later QK projections, earlier-chunk output projections)
                # pumped into the attention stream at fine grain so the PE
                # never idles while ACT drains exp backlogs. Queue order
                # guarantees QK(c) completes before attn(c) needs it.
                filler = []   # items: (deadline_chunk, closure)

                def pump(n):
                    for _ in range(n):
                        if filler:
                            filler.pop(0)[1]()

                def drain(c):
                    # force-emit everything attn(c) depends on (QK(c), V
                    # s-blocks of chunk c) that the pumps haven't covered.
                    due = [f for d, f in filler if d <= c]
                    filler[:] = [(d, f) for d, f in filler if d > c]
                    for f in due:
                        f()

                def attn_unit(c, hh, npump, last=False):
                    nblk = (c + 1) * (CH // P)
                    HW2 = CH // 2
                    # AV accumulators split by column half: otA covers tq
                    # cols [0:256] of the chunk and closes at s-block
                    # nblk-3 (later diagonal blocks only write cols >= 256),
                    # otB covers [256:512] and closes at nblk-1. Each holds
                    # both heads so a half is one PSUM bank.
                    otA = ps_o.tile([DH + 1, 2, HW2], F32, tag="otA")
                    otB = ps_o.tile([DH + 1, 2, HW2], F32, tag="otB")
                    # zero via DVE, then accumulate with start=False
                    # throughout: a start=True would clear the whole bank,
                    # wiping the other head sharing it.
                    nc.vector.memset(otA[:], 0.0)
                    nc.vector.memset(otB[:], 0.0)
                    pend = None

                    def flush_av(pend):
                        e_sb, i, f0 = pend
                        for h2 in range(2):
                            lhsT = v_aug[:, i, hh,
                                         h2 * (DH + 1):(h2 + 1) * (DH + 1)]
                            if f0 < HW2:
                                nc.tensor.matmul(
                                    otA[:, h2, f0:HW2],
                                    lhsT=lhsT,
                                    rhs=e_sb[:, h2 * CH + f0:h2 * CH + HW2],
                                    start=False,
                                    stop=(i == nblk - 3 and h2 == 1),
                                    skip_group_check=True,
                                )
                                nc.tensor.matmul(
                                    otB[:, h2, :],
                                    lhsT=lhsT,
                                    rhs=e_sb[:, h2 * CH + HW2:(h2 + 1) * CH],
                                    start=False,
                                    stop=(i == nblk - 1 and h2 == 1),
                                    skip_group_check=True,
                                )
                            else:
                                nc.tensor.matmul(
                                    otB[:, h2, f0 - HW2:HW2],
                                    lhsT=lhsT,
                                    rhs=e_sb[:, h2 * CH + f0:(h2 + 1) * CH],
                                    start=False,
                                    stop=(i == nblk - 1 and h2 == 1),
                                    skip_group_check=True,
                                )

                    # normalize, engine-interleaved h2 chains, h2=1 first
                    # (its partition-shifting SBUF->SBUF DMA gates readers of
                    # oT_sb); h2=0 writes straight into oT_sb (partitions
                    # 0-63 line up). Column-split: cols [0:384] are final one
                    # s-block early (the last diagonal block only writes
                    # [384:512]), so the bulk of the chain overlaps the last
                    # AV and only a narrow chain trails it.
                    recip = [sml.tile([1, CH], F32, tag="recip",
                                      name=f"recip{h2}")
                             for h2 in range(2)]
                    bc = [sml.tile([DH, CH], F32, tag="bc", name=f"bc{h2}")
                          for h2 in range(2)]

                    def emit_norm(tile, lo):
                        hi = lo + HW2
                        for h2 in (1, 0):
                            nc.vector.reciprocal(
                                recip[h2][:, lo:hi],
                                tile[DH:DH + 1, h2, :])
                        for h2 in (1, 0):
                            nc.gpsimd.partition_broadcast(
                                bc[h2][:, lo:hi], recip[h2][:, lo:hi])
                        o_n = sml.tile([DH, CH], BF16, tag="on")
                        nc.vector.tensor_mul(
                            out=o_n[:, lo:hi], in0=tile[0:DH, 1, :],
                            in1=bc[1][:, lo:hi])
                        nc.scalar.dma_start(
                            oT_sb[DH:P, hh, c * CH + lo:c * CH + hi],
                            o_n[:, lo:hi])
                        nc.vector.tensor_mul(
                            out=oT_sb[0:DH, hh, c * CH + lo:c * CH + hi],
                            in0=tile[0:DH, 0, :], in1=bc[0][:, lo:hi])

                    pump_at = set()
                    if npump:
                        for j in range(npump - 1):
                            pump_at.add(2 + (j * max(nblk - 2, 1))
                                        // max(npump - 1, 1))
                    for i in range(nblk):
                        r = i - c * (CH // P)
                        f0 = P * r if r >= 0 else 0
                        s_ps = ps_s.tile([P, 2 * CH], F32, tag="s")
                        e_sb = expp.tile([P, 2 * CH], BF16, tag="e")
                        for h2 in range(2):
                            pb = h2 * DH
                            nc.tensor.matmul(
                                s_ps[:, h2 * CH + f0:(h2 + 1) * CH],
                                lhsT=kt[pb:pb + DH, hh, i * P:(i + 1) * P],
                                rhs=qt[pb:pb + DH, hh,
                                       c * CH + f0:(c + 1) * CH],
                                start=True, stop=True,
                            )
                        if r >= 0:
                            for h2 in range(2):
                                nc.tensor.matmul(
                                    s_ps[:, h2 * CH + f0:h2 * CH + f0 + P],
                                    lhsT=id_sb[:],
                                    rhs=tril_sb[:],
                                    start=False, stop=True,
                                    skip_group_check=True,
                                )
                        # one exp for both heads: strided [P, 2, CH-f0]
                        s_v = s_ps.rearrange(
                            "p (h w) -> p h w", h=2)[:, :, f0:CH]
                        e_v = e_sb.rearrange(
                            "p (h w) -> p h w", h=2)[:, :, f0:CH]
                        nc.scalar.activation(e_v, s_v, EXP, scale=SCALE)
                        if pend is not None:
                            flush_av(pend)
                            if pend[1] == nblk - 3:
                                emit_norm(otA, 0)
                        if i in pump_at:
                            pump(1)
                        pend = (e_sb, i, f0)

                    pump(1)   # cover the last exp's latency
                    flush_av(pend)
                    if last:
                        pump(len(filler))
                    emit_norm(otB, HW2)

                # Pump rates sized so the queue drains exactly when each
                # item is last needed: QK(c+1) + V(chunk c+1) before
                # attn(c+1), OP(c-1) anytime after attn(c-1).
                NPUMP = npump or {0: 1, 1: 4, 2: 4, 3: 4}
                for c in range(NCH):
                    if c + 1 < NCH:
                        for hh in range(NPAIR):
                            filler.append((c + 1, lambda c4=c + 1, hh=hh:
                                           qk_unit(c4, hh, wq_sb, qt)))
                            filler.append((c + 1, lambda c4=c + 1, hh=hh:
                                           qk_unit(c4, hh, wk_sb, kt)))
                        for sb_i in range(4 * (c + 1), 4 * (c + 2)):
                            filler.append((c + 1, lambda sb=sb_i: v_unit(sb)))
                    if c >= 1:
                        for u in range(2 * NPAIR):
                            filler.append((NCH, lambda cc=c - 1, u=u:
                                           op_unit(cc, u)))
                    drain(c)
                    for hh in range(NPAIR):
                        attn_unit(c, hh, NPUMP[c],
                                  last=(c == NCH - 1 and hh == NPAIR - 1))

                pump(len(filler))
                for u in range(2 * NPAIR):
                    op_unit(NCH - 1, u)

    nc.compile()
    return nc


def _get_nc():
    if 'nc' not in _CACHE:
        _CACHE['nc'] = build_nc()
    return _CACHE['nc']


def _make_in_maps(inputs):
    import ml_dtypes
    bf16 = ml_dtypes.bfloat16
    x = np.asarray(inputs["x"], dtype=np.float32)
    W_qkv = np.asarray(inputs["W_qkv"], dtype=np.float32)
    W_out = np.asarray(inputs["W_out"], dtype=np.float32)

    tril_m = np.where(
        np.arange(P)[:, None] <= np.arange(P)[None, :], 0.0, -1e30
    ).astype(bf16)
    ident = np.eye(P, dtype=bf16)

    in_maps = []
    for core in range(8):
        b, g = core // 2, core % 2
        in_maps.append({
            "xT": np.ascontiguousarray(x[b].T).astype(bf16),
            "wq": np.ascontiguousarray(
                W_qkv[:, g * GD:(g + 1) * GD]).astype(bf16),
            "wk": np.ascontiguousarray(
                W_qkv[:, D + g * GD:D + (g + 1) * GD]).astype(bf16),
            "wv": np.ascontiguousarray(
                W_qkv[:, 2 * D + g * GD:2 * D + (g + 1) * GD]).astype(bf16),
            "wout": np.ascontiguousarray(
                W_out[g * GD:(g + 1) * GD, :]).astype(bf16),
            "tril": tril_m,
            "ident": ident,
        })
    return in_maps


def kernel(x, W_qkv, W_out, mask):
    """Full inputs in, full output out. mask is the known causal tril."""
    in_maps = _make_in_maps({"x": x, "W_qkv": W_qkv, "W_out": W_out})
    nc = _get_nc()
    try:
        res = run_bass_kernel_spmd(nc, in_maps, core_ids=list(range(8)))
    except Exception:
        import time as _time
        _time.sleep(30)   # transient axon-tunnel hiccups; one retry
        res = run_bass_kernel_spmd(nc, in_maps, core_ids=list(range(8)))

    out = np.empty((B, T, D), dtype=np.float32)
    for b in range(B):
        np.add(res.results[2 * b]["y"], res.results[2 * b + 1]["y"],
               out=out[b])
    return out
